# revision 26
# baseline (speedup 1.0000x reference)
"""Trainium2 Bass kernel for nn_Encoder_HieStackedCorr (fused, bf16).

Math (per batch element, Vmat [N=256, V=2048]):
  W1 = weight_norm(U1_v, U1_g); W2 = weight_norm(U2_v, U2_g)   (host, O(params))
  rightT = relu(W1 @ Vmat.T + b1)   [LR, N]
  leftT  = relu(W2 @ Vmat.T + b2)   [LR, N]
  diag[n] = sum_k leftT[k,n]*rightT[k,n];  d = rsqrt(diag + 1e-6)
  s[k] = sum_n d[n] leftT[k,n]
  t[m] = sum_k s[k] rightT[k,m]
  c[m] = (1 + 1/N) - d[m]*t[m]/N          (= mean_n of the uncorr matrix)
  featsT[v] = sum_m Vmat[m,v] c[m]        (accumulated transposed, [V])
  x = feats @ W_lin.T                     [B, E]  (fused in same NEFF)
  (b_lin + train-mode BatchNorm epilogue on host, O(B*E))

Sharding: data-parallel over batch B=64 across 8 cores (8 per core);
all params replicated. Each core returns x_shard [8, 1024]; host
gathers and applies the exact batch-global BatchNorm.

dtypes: Vmat / weights are cast to bf16 on host (halves DMA, and PE
runs 1 cycle/row instead of fp32's 4). PSUM accumulation is fp32; the
small per-batch matmuls (diag/broadcast/t/cp-transpose) stay fp32 for
accuracy.

Sync discipline: walrus allows at most ONE sync-wait per engine
instruction (extra waits become standalone EVENT_SEMAPHORE instrs).
Cross-engine clocks are advanced explicitly:
  - PE observes other engines via dummy `ldweights` reads ("sink").
  - DVE/ACT observe other engines via tiny copies into one-off
    never-reused [1,1] tiles ("touch").
With every foreign tick pre-observed, each real instruction carries at
most one wait (usually its own-engine slot-WAW or one data sem).
"""

import numpy as np
from contextlib import ExitStack

import ml_dtypes
import concourse.bass as bass
import concourse.bacc as bacc
import concourse.tile as tile
from concourse import mybir
from concourse.bass_utils import run_bass_kernel_spmd

B, N, V, LR, E = 64, 256, 2048, 64, 1024
NCORES = 8
BC = B // NCORES          # batches per core
NCH = V // 128            # 16 v-chunks
MH = N // 128             # 2 m-chunks of n/m axis
F32 = mybir.dt.float32
BF16 = mybir.dt.bfloat16


def build_kernel():
    nc = bacc.Bacc()
    vm = nc.declare_dram_parameter("vm", [BC, N, V], BF16, isOutput=False)
    wcombT = nc.declare_dram_parameter("wcombT", [V, 128], BF16, isOutput=False)
    bcomb = nc.declare_dram_parameter("bcomb", [128, 1], F32, isOutput=False)
    wlinT = nc.declare_dram_parameter("wlinT", [V, E], BF16, isOutput=False)
    xout = nc.declare_dram_parameter("xout", [BC, E], F32, isOutput=True)

    with tile.TileContext(nc) as tc:
        _body(tc, vm, wcombT, bcomb, wlinT, xout)
    nc.finalize()
    return nc


def _body(tc, vm, wcombT, bcomb, wlinT, xout):
    nc = tc.nc

    with ExitStack() as ctx:
        consts = ctx.enter_context(tc.tile_pool(name="consts", bufs=1))
        ident = consts.tile([128, 128], F32)
        nc.gpsimd.memset(ident, 0.0)
        nc.gpsimd.affine_select(
            out=ident, in_=ident,
            compare_op=mybir.AluOpType.not_equal,
            fill=1.0, base=0, pattern=[[-1, 128]], channel_multiplier=1,
        )
        ident_bf = consts.tile([128, 128], BF16)
        nc.vector.tensor_copy(out=ident_bf, in_=ident)
        ones_col = consts.tile([128, 1], F32)
        nc.vector.memset(ones_col, 1.0)
        ones_row = consts.tile([1, 128], F32)
        nc.vector.memset(ones_row, 1.0)
        eps_col = consts.tile([128, 1], F32)
        nc.vector.memset(eps_col, 1e-6)
        eps_t = consts.tile([1, 1], F32)
        nc.vector.memset(eps_t, 1e-6)
        # consts ride the scalar HWDGE queue so the sync queue leads with
        # the batch-0 Vmat load (startup latency)
        bcomb_sb = consts.tile([128, 1], F32)
        nc.scalar.dma_start(out=bcomb_sb, in_=bcomb[:, :])
        wcomb_sb = consts.tile([128, NCH, 128], BF16)
        nc.scalar.dma_start(
            out=wcomb_sb, in_=wcombT.rearrange("(c p) k -> p c k", p=128)
        )
        wlin_sb = consts.tile([128, NCH, E], BF16)  # loaded late, sync queue
        featsT_sb = consts.tile([128, NCH, BC], BF16)

        vmat_pool = ctx.enter_context(tc.tile_pool(name="vmat", bufs=8))
        vt_pool = ctx.enter_context(tc.tile_pool(name="vt", bufs=16))
        work = ctx.enter_context(tc.tile_pool(name="work", bufs=2))
        tpool = ctx.enter_context(tc.tile_pool(name="touch", bufs=1))
        tcnt = [0]

        def sink(ap):
            """PE observes ap's producer: dummy ldweights (no output, 1 wait)."""
            nc.tensor.ldweights(ap if ap.dtype == BF16 else ap.bitcast(BF16))

        def dve_touch(ap):
            """DVE observes ap's producer: tiny copy into a one-off tile."""
            tcnt[0] += 1
            t = tpool.tile([1, 1], F32, name=f"tch{tcnt[0]}", tag=f"tch{tcnt[0]}")
            nc.vector.tensor_copy(out=t, in_=ap)

        def act_touch(ap):
            """ACT observes ap's producer: tiny copy into a one-off tile."""
            tcnt[0] += 1
            t = tpool.tile([1, 1], F32, name=f"tch{tcnt[0]}", tag=f"tch{tcnt[0]}")
            nc.scalar.activation(
                out=t, in_=ap, func=mybir.ActivationFunctionType.Copy
            )

        pdf_ctx = ExitStack()
        proj_ps = pdf_ctx.enter_context(
            tc.tile_pool(name="proj_ps", bufs=2, space="PSUM"))
        tp_ps_pool = pdf_ctx.enter_context(
            tc.tile_pool(name="tp_ps", bufs=2, space="PSUM"))
        d_ps_pool = pdf_ctx.enter_context(
            tc.tile_pool(name="d_ps", bufs=1, space="PSUM"))
        f_ps_pool = pdf_ctx.enter_context(
            tc.tile_pool(name="f_ps", bufs=2, space="PSUM"))

        # absorb const-producer waits before use
        sink(ident_bf[0:1, 0:1])        # PE observes DVE (ident cast)
        sink(wcomb_sb[0:1, 0, 0:1])     # PE observes scalar DMA queue
        act_touch(bcomb_sb[0:1, 0:1])   # ACT observes bcomb DMA queue
        act_touch(eps_t[0:1, 0:1])      # ACT observes DVE (eps memset)

        def load_vmat(b):
            """Partition p holds Vmat rows 2p (h=0) and 2p+1 (h=1): adjacent
            DRAM rows are one contiguous 8KB span, so each queue generates one
            descriptor per partition (descriptor GENERATION on the issuing
            engine paces DMA, ~60ns/desc). The induced m-permutation
            (m = 2p+h instead of h*128+p) cancels algebraically: every
            consumer (transposes -> psp columns -> diag/t/c chain -> feats
            matmuls) indexes m through the same (p, h) basis."""
            vmt = vmat_pool.tile([128, MH, V], BF16, tag="vmt")
            r = vm[b].rearrange("(p h) v -> p h v", p=128)
            nc.sync.dma_start(out=vmt[:, 0, :], in_=r[:, 0, :])
            nc.scalar.dma_start(out=vmt[:, 1, :], in_=r[:, 1, :])
            return vmt

        def proj_phase(b, vmt, prev_sq):
            """Transposes + projection matmuls for batch b. Returns psum [128, N]:
            rows 0:64 = rightT, 64:128 = leftT (pre-bias, pre-relu).
            PSUM->SBUF vt copies alternate DVE/ACT to split the load."""
            psp_full = proj_ps.tile([128, 512], F32, tag="psp")
            psp = psp_full[:, 0:N]
            sink(vmt[0:1, 0, 0:1])  # PE observes vmt h0 DMA (sync queue)
            sink(vmt[0:1, 1, 0:1])  # PE observes vmt h1 DMA (scalar queue)
            prev = None  # (chunk_idx, vt_sb)
            for c in range(NCH):
                if c == 1 and prev_sq is not None:
                    # PE observes ACT >= sqrt(b-2) (covers relu/relu2(b-2)
                    # reads that released this psp slot)
                    sink(prev_sq[0:1, 0:1])
                vt_p = tp_ps_pool.tile([128, N], BF16, tag="vt_p")
                for h in range(MH):
                    nc.tensor.transpose(
                        out=vt_p[:, h * 128 : (h + 1) * 128],
                        in_=vmt[:, h, c * 128 : (c + 1) * 128],
                        identity=ident_bf,
                    )
                if c == 0:
                    dve_touch(vt_p[0:1, 0:1])  # DVE observes PE for batch b
                vt_sb = vt_pool.tile([128, N], BF16, tag="vt_sb")
                if c % 2 == 0:
                    nc.vector.tensor_copy(out=vt_sb, in_=vt_p)
                else:
                    # split the copy load: DVE also carries the df chain, ACT
                    # is light; alternating keeps proj matmuls fed by whichever
                    # engine is free
                    nc.scalar.activation(
                        out=vt_sb, in_=vt_p,
                        func=mybir.ActivationFunctionType.Copy,
                    )
                if prev is not None:
                    pc, pvt = prev
                    nc.tensor.matmul(
                        out=psp, lhsT=wcomb_sb[:, pc, :], rhs=pvt,
                        start=(pc == 0), stop=False,
                    )
                prev = (c, vt_sb)
            pc, pvt = prev
            nc.tensor.matmul(
                out=psp, lhsT=wcomb_sb[:, pc, :], rhs=pvt,
                start=(pc == 0), stop=True,
            )
            return psp

        def df_phase(b, vmt, psp, prev_cp):
            """Per-batch vector math + transposed feats accumulation into
            featsT_sb column b. Returns (sq_sb, cp_sb)."""
            act_touch(psp[0:1, 0:1])            # ACT observes PE(psp)
            if prev_cp is not None:
                # ACT observes DVE >= cp-copy(b-1): releases of this batch's
                # d_ps rotation slots are all older DVE/ACT reads
                act_touch(prev_cp[0:1, 0:1])
            # relu'd right into PSUM first, so the later left*right product
            # can mix spaces (base-partition equality only binds SBUF pairs)
            rr_ps = d_ps_pool.tile([64, N], F32, tag="dps")
            nc.scalar.activation(
                out=rr_ps, in_=psp[0:64, :],
                func=mybir.ActivationFunctionType.Relu,
                bias=bcomb_sb[0:64, :], scale=1.0,
            )
            lr_sb = work.tile([128, N], F32, tag="lr")
            nc.scalar.activation(
                out=lr_sb, in_=psp, func=mybir.ActivationFunctionType.Relu,
                bias=bcomb_sb, scale=1.0,
            )
            rightT = lr_sb[0:64, :]
            leftT = lr_sb[64:128, :]
            sink(lr_sb[0:1, 0:1])               # PE observes ACT >= relu > rr
            dve_touch(lr_sb[0:1, 0:1])          # DVE observes ACT(relu)
            dve_touch(rr_ps[0:1, 0:1])          # DVE observes ACT(relu2)
            lrprod = work.tile([64, N], F32, tag="lrprod")
            nc.vector.tensor_mul(lrprod, leftT, rr_ps)
            sink(lrprod[0:1, 0:1])              # PE observes DVE(lrprod)
            # diag/d/t/c run in TRANSPOSED [128, MH] layout: the reciprocal
            # then uses 128 lanes (~110ns) instead of one (~1.7us), and c is
            # born in the cp layout the feats matmuls need (no cp transposes)
            diagT_ps = d_ps_pool.tile([128, MH], F32, tag="dps")
            for h in range(MH):
                nc.tensor.matmul(
                    out=diagT_ps[:, h : h + 1],
                    lhsT=lrprod[:, h * 128 : (h + 1) * 128],
                    rhs=ones_col[0:64, :],
                    start=True, stop=True,
                )
            act_touch(diagT_ps[0:1, 0:1])       # ACT observes PE(diagT)
            sq_sb = work.tile([128, MH], F32, tag="sq")
            nc.scalar.activation(
                out=sq_sb, in_=diagT_ps, func=mybir.ActivationFunctionType.Sqrt,
                bias=eps_col, scale=1.0,
            )
            dve_touch(sq_sb[0:1, 0:1])          # DVE observes ACT(sqrt)
            dT_sb = work.tile([128, MH], F32, tag="d")
            nc.vector.reciprocal(out=dT_sb, in_=sq_sb)
            sink(sq_sb[0:1, 0:1])               # PE observes ACT(sqrt)
            sink(dT_sb[0:1, 0:1])               # PE observes DVE(recip)
            # d back to row form for the 64-partition broadcast
            drow_ps = d_ps_pool.tile([1, N], F32, tag="dps")
            for h in range(MH):
                nc.tensor.transpose(
                    out=drow_ps[0:1, h * 128 : (h + 1) * 128],
                    in_=dT_sb[:, h : h + 1],
                    identity=ident,
                )
            dve_touch(drow_ps[0:1, 0:1])        # DVE observes PE(drow)
            drow_sb = work.tile([1, N], F32, tag="drow")
            nc.vector.tensor_copy(out=drow_sb, in_=drow_ps)
            sink(drow_sb[0:1, 0:1])             # PE observes DVE(drow copy)
            dbc_ps = d_ps_pool.tile([64, N], F32, tag="dps")
            nc.tensor.matmul(
                out=dbc_ps, lhsT=ones_row[0:1, 0:64], rhs=drow_sb,
                start=True, stop=True,
            )
            dve_touch(dbc_ps[0:1, 0:1])         # DVE observes PE(dbc)
            dleft = work.tile([64, N], F32, tag="dleft")
            nc.vector.tensor_mul(dleft, leftT, dbc_ps)
            s_sb = work.tile([64, 1], F32, tag="s")
            nc.vector.reduce_sum(out=s_sb, in_=dleft, axis=mybir.AxisListType.X)
            sink(s_sb[0:1, 0:1])                # PE observes DVE(reduce)
            tT_ps = d_ps_pool.tile([128, MH], F32, tag="dps")
            for h in range(MH):
                nc.tensor.matmul(
                    out=tT_ps[:, h : h + 1],
                    lhsT=rightT[:, h * 128 : (h + 1) * 128],
                    rhs=s_sb,
                    start=True, stop=True,
                )
            dve_touch(tT_ps[0:1, 0:1])          # DVE observes PE(tT)
            dtT_sb = work.tile([128, MH], F32, tag="dt")
            nc.vector.tensor_mul(dtT_sb, dT_sb, tT_ps)
            cp_sb = work.tile([128, MH], BF16, tag="cp")
            nc.vector.tensor_scalar(
                out=cp_sb, in0=dtT_sb, scalar1=-1.0 / N, scalar2=1.0 + 1.0 / N,
                op0=mybir.AluOpType.mult, op1=mybir.AluOpType.add,
            )
            sink(cp_sb[0:1, 0:1])               # PE observes DVE(cp)
            # featsT[:, c, b] = sum_h vmt[:, h, c*128:(c+1)*128].T @ cp[:, h]
            # (1-row matmuls: Vmat chunk is the stationary operand)
            fT_ps = f_ps_pool.tile([128, NCH], F32, tag="fps")
            for cch in range(NCH):
                for h in range(MH):
                    nc.tensor.matmul(
                        out=fT_ps[:, cch : cch + 1],
                        lhsT=vmt[:, h, cch * 128 : (cch + 1) * 128],
                        rhs=cp_sb[:, h : h + 1],
                        start=(h == 0), stop=(h == MH - 1),
                    )
            dve_touch(fT_ps[0:1, 0:1])          # DVE observes PE(featsT)
            nc.vector.tensor_copy(out=featsT_sb[:, :, b], in_=fT_ps)
            return sq_sb, cp_sb

        # ---- software-pipelined batch loop: proj(b) runs while DF(b-1) drains
        vmt_prev = load_vmat(0)
        psp_prev = None
        sq_hist = [None, None]  # sq_sb handles of df(b-1), df(b-2)
        cp_prev = None
        sq_last = None
        for b in range(BC):
            psp = proj_phase(b, vmt_prev, sq_hist[1])
            vmt_cur = vmt_prev
            if b + 1 < BC:
                vmt_next = load_vmat(b + 1)
            if b == 5:
                # final-projection weights on the sync queue once the vmat
                # load cadence has slack; needed only by the B-phase tail
                nc.sync.dma_start(
                    out=wlin_sb,
                    in_=wlinT.rearrange("(c p) e -> p c e", p=128),
                )
            if psp_prev is not None:
                sq_i, cp_prev = df_phase(b - 1, vmt_pp, psp_prev, cp_prev)
                sq_hist = [sq_i, sq_hist[0]]
            psp_prev, vmt_pp = psp, vmt_cur
            if b + 1 < BC:
                vmt_prev = vmt_next
        sq_last, _ = df_phase(BC - 1, vmt_pp, psp_prev, cp_prev)

        # ---- fused final projection: x = feats @ W_lin.T  [BC, E]
        # PE pre-observes every engine so the B-phase matmuls run wait-free
        sink(featsT_sb[0:1, NCH - 1, BC - 1 : BC])  # DVE >= featsT copy(b=7)
        sink(sq_last[0:1, 0:1])                     # ACT >= sqrt(b=7)
        sink(wlin_sb[0:1, 0, 0:1])                  # sync-q >= wlin DMA
        pdf_ctx.close()
        bctx = ExitStack()
        xps_pool = bctx.enter_context(
            tc.tile_pool(name="x_ps", bufs=1, space="PSUM"))
        x_ps = xps_pool.tile([BC, E], F32, tag="xps")
        for c in range(NCH):
            for seg in range(E // 512):
                nc.tensor.matmul(
                    out=x_ps[:, seg * 512 : (seg + 1) * 512],
                    lhsT=featsT_sb[:, c, :],
                    rhs=wlin_sb[:, c, seg * 512 : (seg + 1) * 512],
                    start=(c == 0), stop=(c == NCH - 1),
                )
        x_sb = consts.tile([BC, E], F32)
        nc.scalar.activation(
            out=x_sb, in_=x_ps, func=mybir.ActivationFunctionType.Copy
        )
        nc.gpsimd.dma_start(out=xout[:, :], in_=x_sb)
        bctx.close()


_NC_CACHE = {}

# test-harness knobs (ignored by graders calling kernel() directly)
PROFILE = False
LAST_RESULT = None
LAST_RESULT_B = None


def _get_nc():
    if "k" not in _NC_CACHE:
        _NC_CACHE["k"] = build_kernel()
    return _NC_CACHE["k"]


def kernel(**inputs):
    Vmat = np.asarray(inputs["Vmat"], dtype=np.float32)
    U1_v = np.asarray(inputs["U1_v"], dtype=np.float32)
    U1_g = np.asarray(inputs["U1_g"], dtype=np.float32)
    U1_b = np.asarray(inputs["U1_b"], dtype=np.float32)
    U2_v = np.asarray(inputs["U2_v"], dtype=np.float32)
    U2_g = np.asarray(inputs["U2_g"], dtype=np.float32)
    U2_b = np.asarray(inputs["U2_b"], dtype=np.float32)
    W_lin = np.asarray(inputs["W_lin"], dtype=np.float32)
    b_lin = np.asarray(inputs["b_lin"], dtype=np.float32)
    bn_gamma = np.asarray(inputs["bn_gamma"], dtype=np.float32)
    bn_beta = np.asarray(inputs["bn_beta"], dtype=np.float32)

    # host O(params) prep: weight-norm + packed transposed bf16 layouts
    W1 = U1_v * (U1_g / np.linalg.norm(U1_v, axis=1))[:, None]
    W2 = U2_v * (U2_g / np.linalg.norm(U2_v, axis=1))[:, None]
    wcombT = np.ascontiguousarray(
        np.concatenate([W1, W2], axis=0).T
    ).astype(ml_dtypes.bfloat16)  # [V, 128]
    bcomb = np.concatenate([U1_b, U2_b]).reshape(128, 1).astype(np.float32)
    wlinT = np.ascontiguousarray(W_lin.T).astype(ml_dtypes.bfloat16)  # [V, E]
    vm_bf = Vmat.astype(ml_dtypes.bfloat16)  # [B, N, V]

    nc = _get_nc()
    in_maps = [
        {
            "vm": vm_bf[i * BC : (i + 1) * BC],
            "wcombT": wcombT,
            "bcomb": bcomb,
            "wlinT": wlinT,
        }
        for i in range(NCORES)
    ]
    global LAST_RESULT, LAST_RESULT_B
    res = run_bass_kernel_spmd(nc, in_maps, list(range(NCORES)), trace=PROFILE)
    LAST_RESULT = res
    LAST_RESULT_B = None
    x = np.concatenate(
        [np.asarray(res.results[i]["xout"]) for i in range(NCORES)], axis=0
    )

    # exact batch-global BatchNorm epilogue (b_lin cancels but keep fidelity)
    x = x + b_lin
    mu = x.mean(axis=0)
    var = np.mean((x - mu) ** 2, axis=0)
    out = bn_gamma * (x - mu) / np.sqrt(var + 1e-5) + bn_beta
    return out.astype(np.float32)


# revision 32
# speedup vs baseline: 1.0416x; 1.0416x over previous
"""Trainium2 Bass kernel for nn_Encoder_HieStackedCorr (fused, bf16).

Math (per batch element, Vmat [N=256, V=2048]):
  W1 = weight_norm(U1_v, U1_g); W2 = weight_norm(U2_v, U2_g)   (host, O(params))
  rightT = relu(W1 @ Vmat.T + b1)   [LR, N]
  leftT  = relu(W2 @ Vmat.T + b2)   [LR, N]
  diag[n] = sum_k leftT[k,n]*rightT[k,n];  d = rsqrt(diag + 1e-6)
  s[k] = sum_n d[n] leftT[k,n]
  t[m] = sum_k s[k] rightT[k,m]
  c[m] = (1 + 1/N) - d[m]*t[m]/N          (= mean_n of the uncorr matrix)
  featsT[v] = sum_m Vmat[m,v] c[m]        (accumulated transposed, [V])
  x = feats @ W_lin.T                     [B, E]  (fused in same NEFF)
  (b_lin + train-mode BatchNorm epilogue on host, O(B*E))

Sharding: data-parallel over batch B=64 across 8 cores (8 per core);
all params replicated. Each core returns x_shard [8, 1024]; host
gathers and applies the exact batch-global BatchNorm.

dtypes: Vmat / weights are cast to bf16 on host (halves DMA, and PE
runs 1 cycle/row instead of fp32's 4). PSUM accumulation is fp32; the
small per-batch matmuls (diag/broadcast/t/cp-transpose) stay fp32 for
accuracy.

Sync discipline: walrus allows at most ONE sync-wait per engine
instruction (extra waits become standalone EVENT_SEMAPHORE instrs).
Cross-engine clocks are advanced explicitly:
  - PE observes other engines via dummy `ldweights` reads ("sink").
  - DVE/ACT observe other engines via tiny copies into one-off
    never-reused [1,1] tiles ("touch").
With every foreign tick pre-observed, each real instruction carries at
most one wait (usually its own-engine slot-WAW or one data sem).
"""

import numpy as np
from contextlib import ExitStack

import ml_dtypes
import concourse.bass as bass
import concourse.bacc as bacc
import concourse.tile as tile
from concourse import mybir
from concourse.bass_utils import run_bass_kernel_spmd

B, N, V, LR, E = 64, 256, 2048, 64, 1024
NCORES = 8
BC = B // NCORES          # batches per core
NCH = V // 128            # 16 v-chunks
MH = N // 128             # 2 m-chunks of n/m axis
F32 = mybir.dt.float32
BF16 = mybir.dt.bfloat16


def build_kernel():
    nc = bacc.Bacc()
    vm = nc.declare_dram_parameter("vm", [BC, N, V], BF16, isOutput=False)
    wcombT = nc.declare_dram_parameter("wcombT", [V, 128], BF16, isOutput=False)
    bcomb = nc.declare_dram_parameter("bcomb", [128, 1], F32, isOutput=False)
    wlinT = nc.declare_dram_parameter("wlinT", [V, E], BF16, isOutput=False)
    xout = nc.declare_dram_parameter("xout", [BC, E], F32, isOutput=True)

    with tile.TileContext(nc) as tc:
        _body(tc, vm, wcombT, bcomb, wlinT, xout)
    nc.finalize()
    return nc


def _body(tc, vm, wcombT, bcomb, wlinT, xout):
    nc = tc.nc

    with ExitStack() as ctx:
        consts = ctx.enter_context(tc.tile_pool(name="consts", bufs=1))
        ident = consts.tile([128, 128], F32)
        nc.gpsimd.memset(ident, 0.0)
        nc.gpsimd.affine_select(
            out=ident, in_=ident,
            compare_op=mybir.AluOpType.not_equal,
            fill=1.0, base=0, pattern=[[-1, 128]], channel_multiplier=1,
        )
        ident_bf = consts.tile([128, 128], BF16)
        nc.vector.tensor_copy(out=ident_bf, in_=ident)
        ones_col = consts.tile([128, 1], F32)
        nc.vector.memset(ones_col, 1.0)
        ones_row = consts.tile([1, 128], F32)
        nc.vector.memset(ones_row, 1.0)
        eps_col = consts.tile([128, 1], F32)
        nc.vector.memset(eps_col, 1e-6)
        eps_t = consts.tile([1, 1], F32)
        nc.vector.memset(eps_t, 1e-6)
        # consts ride the scalar HWDGE queue so the sync queue leads with
        # the batch-0 Vmat load (startup latency)
        bcomb_sb = consts.tile([128, 1], F32)
        nc.scalar.dma_start(out=bcomb_sb, in_=bcomb[:, :])
        wcomb_sb = consts.tile([128, NCH, 128], BF16)
        nc.scalar.dma_start(
            out=wcomb_sb, in_=wcombT.rearrange("(c p) k -> p c k", p=128)
        )
        wlin_sb = consts.tile([128, NCH, E], BF16)  # loaded late, sync queue
        feats_sb = consts.tile([BC, V], BF16)
        featsT_sb = consts.tile([128, NCH, BC], BF16)

        vmat_pool = ctx.enter_context(tc.tile_pool(name="vmat", bufs=8))
        vt_pool = ctx.enter_context(tc.tile_pool(name="vt", bufs=16))
        work = ctx.enter_context(tc.tile_pool(name="work", bufs=2))
        tpool = ctx.enter_context(tc.tile_pool(name="touch", bufs=1))
        tcnt = [0]

        def sink(ap):
            """PE observes ap's producer: dummy ldweights (no output, 1 wait)."""
            nc.tensor.ldweights(ap if ap.dtype == BF16 else ap.bitcast(BF16))

        def dve_touch(ap):
            """DVE observes ap's producer: tiny copy into a one-off tile."""
            tcnt[0] += 1
            t = tpool.tile([1, 1], F32, name=f"tch{tcnt[0]}", tag=f"tch{tcnt[0]}")
            nc.vector.tensor_copy(out=t, in_=ap)

        def act_touch(ap):
            """ACT observes ap's producer: tiny copy into a one-off tile."""
            tcnt[0] += 1
            t = tpool.tile([1, 1], F32, name=f"tch{tcnt[0]}", tag=f"tch{tcnt[0]}")
            nc.scalar.activation(
                out=t, in_=ap, func=mybir.ActivationFunctionType.Copy
            )

        pdf_ctx = ExitStack()
        proj_ps = pdf_ctx.enter_context(
            tc.tile_pool(name="proj_ps", bufs=2, space="PSUM"))
        tp_ps_pool = pdf_ctx.enter_context(
            tc.tile_pool(name="tp_ps", bufs=2, space="PSUM"))
        d_ps_pool = pdf_ctx.enter_context(
            tc.tile_pool(name="d_ps", bufs=1, space="PSUM"))
        f_ps_pool = pdf_ctx.enter_context(
            tc.tile_pool(name="f_ps", bufs=2, space="PSUM"))

        # absorb const-producer waits before use
        sink(ident_bf[0:1, 0:1])        # PE observes DVE (ident cast)
        sink(wcomb_sb[0:1, 0, 0:1])     # PE observes scalar DMA queue
        act_touch(bcomb_sb[0:1, 0:1])   # ACT observes bcomb DMA queue
        act_touch(eps_t[0:1, 0:1])      # ACT observes DVE (eps memset)

        def load_vmat(b):
            """Partition p holds Vmat rows 2p (h=0) and 2p+1 (h=1): adjacent
            DRAM rows are one contiguous 8KB span, so each queue generates one
            descriptor per partition (descriptor GENERATION on the issuing
            engine paces DMA, ~60ns/desc). The induced m-permutation
            (m = 2p+h instead of h*128+p) cancels algebraically: every
            consumer (transposes -> psp columns -> diag/t/c chain -> feats
            matmuls) indexes m through the same (p, h) basis."""
            vmt = vmat_pool.tile([128, MH, V], BF16, tag="vmt")
            r = vm[b].rearrange("(p h) v -> p h v", p=128)
            nc.sync.dma_start(out=vmt[:, 0, :], in_=r[:, 0, :])
            nc.scalar.dma_start(out=vmt[:, 1, :], in_=r[:, 1, :])
            return vmt

        def proj_phase(b, vmt, prev_sq):
            """Transposes + projection matmuls for batch b. Returns psum [128, N]:
            rows 0:64 = rightT, 64:128 = leftT (pre-bias, pre-relu).
            PSUM->SBUF vt copies alternate DVE/ACT to split the load."""
            psp_full = proj_ps.tile([128, 512], F32, tag="psp")
            psp = psp_full[:, 0:N]
            sink(vmt[0:1, 0, 0:1])  # PE observes vmt h0 DMA (sync queue)
            sink(vmt[0:1, 1, 0:1])  # PE observes vmt h1 DMA (scalar queue)
            prev = None  # (chunk_idx, vt_sb)
            for c in range(NCH):
                if c == 1 and prev_sq is not None:
                    # PE observes ACT >= sqrt(b-2) (covers relu/relu2(b-2)
                    # reads that released this psp slot)
                    sink(prev_sq[0:1, 0:1])
                vt_p = tp_ps_pool.tile([128, N], BF16, tag="vt_p")
                for h in range(MH):
                    nc.tensor.transpose(
                        out=vt_p[:, h * 128 : (h + 1) * 128],
                        in_=vmt[:, h, c * 128 : (c + 1) * 128],
                        identity=ident_bf,
                    )
                if c == 0:
                    dve_touch(vt_p[0:1, 0:1])  # DVE observes PE for batch b
                vt_sb = vt_pool.tile([128, N], BF16, tag="vt_sb")
                nc.vector.tensor_copy(out=vt_sb, in_=vt_p)
                if prev is not None:
                    pc, pvt = prev
                    nc.tensor.matmul(
                        out=psp, lhsT=wcomb_sb[:, pc, :], rhs=pvt,
                        start=(pc == 0), stop=False,
                    )
                prev = (c, vt_sb)
            pc, pvt = prev
            nc.tensor.matmul(
                out=psp, lhsT=wcomb_sb[:, pc, :], rhs=pvt,
                start=(pc == 0), stop=True,
            )
            return psp

        def df_phase(b, vmt, psp, prev_cp):
            """Per-batch vector math + transposed feats accumulation into
            featsT_sb column b. Returns (sq_sb, cp_sb)."""
            act_touch(psp[0:1, 0:1])            # ACT observes PE(psp)
            if prev_cp is not None:
                # ACT observes DVE >= cp-copy(b-1): releases of this batch's
                # d_ps rotation slots are all older DVE/ACT reads
                act_touch(prev_cp[0:1, 0:1])
            # relu'd right into PSUM first, so the later left*right product
            # can mix spaces (base-partition equality only binds SBUF pairs)
            rr_ps = d_ps_pool.tile([64, N], F32, tag="dps")
            nc.scalar.activation(
                out=rr_ps, in_=psp[0:64, :],
                func=mybir.ActivationFunctionType.Relu,
                bias=bcomb_sb[0:64, :], scale=1.0,
            )
            lr_sb = work.tile([128, N], F32, tag="lr")
            nc.scalar.activation(
                out=lr_sb, in_=psp, func=mybir.ActivationFunctionType.Relu,
                bias=bcomb_sb, scale=1.0,
            )
            rightT = lr_sb[0:64, :]
            leftT = lr_sb[64:128, :]
            sink(lr_sb[0:1, 0:1])               # PE observes ACT >= relu > rr
            dve_touch(lr_sb[0:1, 0:1])          # DVE observes ACT(relu)
            dve_touch(rr_ps[0:1, 0:1])          # DVE observes ACT(relu2)
            lrprod = work.tile([64, N], F32, tag="lrprod")
            nc.vector.tensor_mul(lrprod, leftT, rr_ps)
            sink(lrprod[0:1, 0:1])              # PE observes DVE(lrprod)
            # diag/d/t/c run in TRANSPOSED [128, MH] layout: the reciprocal
            # then uses 128 lanes (~110ns) instead of one (~1.7us), and c is
            # born in the cp layout the feats matmuls need (no cp transposes)
            diagT_ps = d_ps_pool.tile([128, MH], F32, tag="dps")
            for h in range(MH):
                nc.tensor.matmul(
                    out=diagT_ps[:, h : h + 1],
                    lhsT=lrprod[:, h * 128 : (h + 1) * 128],
                    rhs=ones_col[0:64, :],
                    start=True, stop=True,
                )
            act_touch(diagT_ps[0:1, 0:1])       # ACT observes PE(diagT)
            sq_sb = work.tile([128, MH], F32, tag="sq")
            nc.scalar.activation(
                out=sq_sb, in_=diagT_ps, func=mybir.ActivationFunctionType.Sqrt,
                bias=eps_col, scale=1.0,
            )
            dve_touch(sq_sb[0:1, 0:1])          # DVE observes ACT(sqrt)
            dT_sb = work.tile([128, MH], F32, tag="d")
            nc.vector.reciprocal(out=dT_sb, in_=sq_sb)
            sink(sq_sb[0:1, 0:1])               # PE observes ACT(sqrt)
            sink(dT_sb[0:1, 0:1])               # PE observes DVE(recip)
            # d back to row form for the 64-partition broadcast
            drow_ps = d_ps_pool.tile([1, N], F32, tag="dps")
            for h in range(MH):
                nc.tensor.transpose(
                    out=drow_ps[0:1, h * 128 : (h + 1) * 128],
                    in_=dT_sb[:, h : h + 1],
                    identity=ident,
                )
            dve_touch(drow_ps[0:1, 0:1])        # DVE observes PE(drow)
            drow_sb = work.tile([1, N], F32, tag="drow")
            nc.vector.tensor_copy(out=drow_sb, in_=drow_ps)
            sink(drow_sb[0:1, 0:1])             # PE observes DVE(drow copy)
            dbc_ps = d_ps_pool.tile([64, N], F32, tag="dps")
            nc.tensor.matmul(
                out=dbc_ps, lhsT=ones_row[0:1, 0:64], rhs=drow_sb,
                start=True, stop=True,
            )
            dve_touch(dbc_ps[0:1, 0:1])         # DVE observes PE(dbc)
            dleft = work.tile([64, N], F32, tag="dleft")
            nc.vector.tensor_mul(dleft, leftT, dbc_ps)
            s_sb = work.tile([64, 1], F32, tag="s")
            nc.vector.reduce_sum(out=s_sb, in_=dleft, axis=mybir.AxisListType.X)
            sink(s_sb[0:1, 0:1])                # PE observes DVE(reduce)
            tT_ps = d_ps_pool.tile([128, MH], F32, tag="dps")
            for h in range(MH):
                nc.tensor.matmul(
                    out=tT_ps[:, h : h + 1],
                    lhsT=rightT[:, h * 128 : (h + 1) * 128],
                    rhs=s_sb,
                    start=True, stop=True,
                )
            dve_touch(tT_ps[0:1, 0:1])          # DVE observes PE(tT)
            dtT_sb = work.tile([128, MH], F32, tag="dt")
            nc.vector.tensor_mul(dtT_sb, dT_sb, tT_ps)
            cp_sb = work.tile([128, MH], BF16, tag="cp")
            nc.vector.tensor_scalar(
                out=cp_sb, in0=dtT_sb, scalar1=-1.0 / N, scalar2=1.0 + 1.0 / N,
                op0=mybir.AluOpType.mult, op1=mybir.AluOpType.add,
            )
            sink(cp_sb[0:1, 0:1])               # PE observes DVE(cp)
            # feats row b: wide [1,512] matmuls with the 1-column cp as the
            # stationary operand (trivial weight load, vs 32 1-row matmuls
            # each paying the ~120ns LDWEIGHTS tax). ACT drains the psum
            # segments into a bf16 staging row; a SWDGE dma shifts it from
            # partition 0 to row b of feats_sb.
            fstage = work.tile([1, V], BF16, tag="fstage")
            for seg in range(V // 512):
                f_ps = f_ps_pool.tile([1, 512], F32, tag="fps")
                for h in range(MH):
                    nc.tensor.matmul(
                        out=f_ps,
                        lhsT=cp_sb[:, h : h + 1],
                        rhs=vmt[:, h, seg * 512 : (seg + 1) * 512],
                        start=(h == 0), stop=(h == MH - 1),
                    )
                nc.scalar.activation(
                    out=fstage[0:1, seg * 512 : (seg + 1) * 512], in_=f_ps,
                    func=mybir.ActivationFunctionType.Copy,
                )
            nc.gpsimd.dma_start(out=feats_sb[b : b + 1, :], in_=fstage)
            return sq_sb, cp_sb, fstage

        # ---- software-pipelined batch loop: proj(b) runs while DF(b-1) drains
        vmt_prev = load_vmat(0)
        psp_prev = None
        sq_hist = [None, None]  # sq_sb handles of df(b-1), df(b-2)
        cp_prev = None
        sq_last = None
        for b in range(BC):
            psp = proj_phase(b, vmt_prev, sq_hist[1])
            vmt_cur = vmt_prev
            if b + 1 < BC:
                vmt_next = load_vmat(b + 1)
            if b == 5:
                # final-projection weights on the sync queue once the vmat
                # load cadence has slack; needed only by the B-phase tail
                nc.sync.dma_start(
                    out=wlin_sb,
                    in_=wlinT.rearrange("(c p) e -> p c e", p=128),
                )
            if psp_prev is not None:
                sq_i, cp_prev, f_last = df_phase(
                    b - 1, vmt_pp, psp_prev, cp_prev
                )
                sq_hist = [sq_i, sq_hist[0]]
            psp_prev, vmt_pp = psp, vmt_cur
            if b + 1 < BC:
                vmt_prev = vmt_next
        sq_last, _, f_last = df_phase(BC - 1, vmt_pp, psp_prev, cp_prev)

        # ---- fused final projection: x = feats @ W_lin.T  [BC, E]
        # PE pre-observes every engine so the B-phase runs with minimal waits
        sink(feats_sb[0:BC, 0:1])                   # swdge-q >= feats DMAs
        sink(f_last[0:1, 0:1])                      # ACT >= fstage copies(b=7)
        sink(wlin_sb[0:1, 0, 0:1])                  # sync-q >= wlin DMA
        pdf_ctx.close()
        bctx = ExitStack()
        xps_pool = bctx.enter_context(
            tc.tile_pool(name="x_ps", bufs=1, space="PSUM"))
        # transpose feats [8, V] chunks into featsT [128, NCH, BC] once
        ftT_ps = xps_pool.tile([128, NCH * BC], BF16, tag="ftp")
        for c in range(NCH):
            nc.tensor.transpose(
                out=ftT_ps[:, c * BC : (c + 1) * BC],
                in_=feats_sb[0:BC, c * 128 : (c + 1) * 128],
                identity=ident_bf[0:BC, 0:BC],
            )
        dve_touch(ftT_ps[0:1, 0:1])                 # DVE observes PE
        nc.vector.tensor_copy(
            out=featsT_sb.rearrange("p c b -> p (c b)"), in_=ftT_ps
        )
        sink(featsT_sb[0:1, NCH - 1, BC - 1 : BC])  # PE observes DVE copy
        x_ps = xps_pool.tile([BC, E], F32, tag="xps")
        for c in range(NCH):
            for seg in range(E // 512):
                nc.tensor.matmul(
                    out=x_ps[:, seg * 512 : (seg + 1) * 512],
                    lhsT=featsT_sb[:, c, :],
                    rhs=wlin_sb[:, c, seg * 512 : (seg + 1) * 512],
                    start=(c == 0), stop=(c == NCH - 1),
                )
        x_sb = consts.tile([BC, E], F32)
        nc.scalar.activation(
            out=x_sb, in_=x_ps, func=mybir.ActivationFunctionType.Copy
        )
        nc.gpsimd.dma_start(out=xout[:, :], in_=x_sb)
        bctx.close()


_NC_CACHE = {}

# test-harness knobs (ignored by graders calling kernel() directly)
PROFILE = False
LAST_RESULT = None
LAST_RESULT_B = None


def _get_nc():
    if "k" not in _NC_CACHE:
        _NC_CACHE["k"] = build_kernel()
    return _NC_CACHE["k"]


def kernel(**inputs):
    Vmat = np.asarray(inputs["Vmat"], dtype=np.float32)
    U1_v = np.asarray(inputs["U1_v"], dtype=np.float32)
    U1_g = np.asarray(inputs["U1_g"], dtype=np.float32)
    U1_b = np.asarray(inputs["U1_b"], dtype=np.float32)
    U2_v = np.asarray(inputs["U2_v"], dtype=np.float32)
    U2_g = np.asarray(inputs["U2_g"], dtype=np.float32)
    U2_b = np.asarray(inputs["U2_b"], dtype=np.float32)
    W_lin = np.asarray(inputs["W_lin"], dtype=np.float32)
    b_lin = np.asarray(inputs["b_lin"], dtype=np.float32)
    bn_gamma = np.asarray(inputs["bn_gamma"], dtype=np.float32)
    bn_beta = np.asarray(inputs["bn_beta"], dtype=np.float32)

    # host O(params) prep: weight-norm + packed transposed bf16 layouts
    W1 = U1_v * (U1_g / np.linalg.norm(U1_v, axis=1))[:, None]
    W2 = U2_v * (U2_g / np.linalg.norm(U2_v, axis=1))[:, None]
    wcombT = np.ascontiguousarray(
        np.concatenate([W1, W2], axis=0).T
    ).astype(ml_dtypes.bfloat16)  # [V, 128]
    bcomb = np.concatenate([U1_b, U2_b]).reshape(128, 1).astype(np.float32)
    wlinT = np.ascontiguousarray(W_lin.T).astype(ml_dtypes.bfloat16)  # [V, E]
    vm_bf = Vmat.astype(ml_dtypes.bfloat16)  # [B, N, V]

    nc = _get_nc()
    in_maps = [
        {
            "vm": vm_bf[i * BC : (i + 1) * BC],
            "wcombT": wcombT,
            "bcomb": bcomb,
            "wlinT": wlinT,
        }
        for i in range(NCORES)
    ]
    global LAST_RESULT, LAST_RESULT_B
    res = run_bass_kernel_spmd(nc, in_maps, list(range(NCORES)), trace=PROFILE)
    LAST_RESULT = res
    LAST_RESULT_B = None
    x = np.concatenate(
        [np.asarray(res.results[i]["xout"]) for i in range(NCORES)], axis=0
    )

    # exact batch-global BatchNorm epilogue (b_lin cancels but keep fidelity)
    x = x + b_lin
    mu = x.mean(axis=0)
    var = np.mean((x - mu) ** 2, axis=0)
    out = bn_gamma * (x - mu) / np.sqrt(var + 1e-5) + bn_beta
    return out.astype(np.float32)
